# revision 27
# baseline (speedup 1.0000x reference)
"""MultiHeadAttention (B=4, S=2048, D=1024, H=16, causal) on 8 TRN2 NeuronCores.

Sharding: tensor-parallel over heads across all 8 cores (2 heads/core, all 4
batches processed locally; identical SPMD control flow on every core). After
attention, per-batch the attention outputs are redistributed by TWO
half-batch AllToAlls (q-columns [0:1024) and [1024:2048)); each core then
runs the output projection for its 128 q-rows of each half. The first half's
collective fires at ~50% of the batch's attention, so the output projection
never head-of-line blocks the in-order PE queue, and the exposed tail after
the last batch is just one 256KB collective + 16 matmuls.

Per-core pipeline (all matmuls bf16 with f32 PSUM accumulation):
  - x arrives host-transposed as x^T [D, B*S] bf16, loaded in two
    single-trigger half-chunks per batch so projection starts ~9us in;
    K^T/Q^T/V^T via w-stationary matmuls, bias added on the DVE eviction.
    V^T is transposed to natural V by PE transposes with a ones column per
    head so the PV matmul also produces the softmax denominator.
  - Scores are computed transposed ([k, q] = K @ Q^T) in 512-wide q-chunks.
    The two heads' score matmuls are issued as row-group tiles (K=64 each,
    rows 0-63 / 64-127) so they can execute concurrently in the PE array and
    land in the two PSUM banks of one [128, 1024] tile; a single wide exp on
    ScalarE covers both heads (1/sqrt(dk) scale folded in; no max subtraction
    needed: |scores| <~ 2.6). Causal mask = 0/1 triangular multiply on
    diagonal tiles only; fully-masked tiles are never computed.
  - PV: [V_h|1] stationary, exp chunks stream, accumulating po_h = [out^T;
    denom] [65, 512] in PSUM, one k-strip behind scores/exp.
  - po is already in the [head-dim, q] layout the AllToAll needs: the
    denominator row is reciprocal'd via a DRAM bounce that spreads it over
    64 lanes, and a single DVE multiply writes the normalized bf16 output
    straight into the half-batch exchange buffer (one DMA per (t, head)).
  - Output projection halves are interleaved into the next batch's
    attention; their aT loads ride the GpSimd queue (already serialized
    behind the collectives) so they never block the sync queue.
"""

import sys

if "/opt/trn_rl_repo" not in sys.path:
    sys.path.insert(0, "/opt/trn_rl_repo")

from contextlib import ExitStack
from itertools import chain

import ml_dtypes
import numpy as np

import concourse.bacc as bacc
import concourse.bass as bass
import concourse.mybir as mybir
import concourse.tile as tile
from concourse.bass_utils import run_bass_kernel_spmd
from concourse.masks import make_identity, make_upper_triangular

N_CORES = 8
B = 4
S = 2048
D = 1024
H_TOT = 16
DK = 64
H_LOC = H_TOT // N_CORES  # 2 heads per core
HC = H_LOC * DK  # 128 head-cols per core
ST = S // 128  # 16 k-strips per batch
DC = D // 128  # 8 d_model chunks
NT = S // 512  # 4 q-chunks per batch
BQ = (B * S) // N_CORES  # 1024 (batch,seq) rows per core after AllToAll

F32 = mybir.dt.float32
BF16 = mybir.dt.bfloat16
BF16_NP = ml_dtypes.bfloat16


def _bcast(handle, rows, cols):
    """AP reading a [1, cols] DRAM tensor broadcast over `rows` partitions."""
    return bass.AP(tensor=handle, offset=0, ap=[[0, rows], [1, cols]])


def build_program():
    nc = bacc.Bacc("TRN2", target_bir_lowering=False, debug=False,
                   num_devices=N_CORES)

    xt = nc.declare_dram_parameter("xt", [128, B * 2 * DC * 1024], BF16,
                               isOutput=False)
    wq = nc.declare_dram_parameter("wq", [128, DC * HC], BF16, isOutput=False)
    wk = nc.declare_dram_parameter("wk", [128, DC * HC], BF16, isOutput=False)
    wv = nc.declare_dram_parameter("wv", [128, DC * HC], BF16, isOutput=False)
    bq = nc.declare_dram_parameter("bq", [HC, 1], F32, isOutput=False)
    bk = nc.declare_dram_parameter("bk", [HC, 1], F32, isOutput=False)
    bv = nc.declare_dram_parameter("bv", [HC, 1], F32, isOutput=False)
    wo = nc.declare_dram_parameter("wo", [128, DC * D], BF16, isOutput=False)
    bo = nc.declare_dram_parameter("bo", [1, D], F32, isOutput=False)
    out = nc.declare_dram_parameter("out", [BQ, D], BF16, isOutput=True)

    with ExitStack() as ctx:
        tc = ctx.enter_context(tile.TileContext(nc))

        consts = ctx.enter_context(tc.tile_pool(name="consts", bufs=1))
        wpool = ctx.enter_context(tc.tile_pool(name="wpool", bufs=1))
        xtp = ctx.enter_context(tc.tile_pool(name="xtp", bufs=2))
        kqv = ctx.enter_context(tc.tile_pool(name="kqv", bufs=2))
        # 5 slots: with PV trailing two strips, es(j)'s allocation must not
        # wait on pv(j-4) execution lag (4 slots is exactly tight)
        epool = ctx.enter_context(tc.tile_pool(name="epool", bufs=5))
        # normalize-chain pools are deep enough (4+ chunks = one full batch)
        # to ride out collective-induced GpSimd-queue blocking without
        # backpressuring the PV/exp pipeline
        rpool = ctx.enter_context(tc.tile_pool(name="rpool", bufs=5))
        bpool = ctx.enter_context(tc.tile_pool(name="bpool", bufs=5))
        spool = ctx.enter_context(tc.tile_pool(name="spool", bufs=10))
        rcpool = ctx.enter_context(tc.tile_pool(name="rcpool", bufs=2))
        # 4 slots: at the batch-3 tail two oproj halves run back-to-back (4
        # osb tiles) while their store DMAs queue behind the final normalize
        # chain on sync — 3 slots would stall the 4th DVE add
        opool = ctx.enter_context(tc.tile_pool(name="opool", bufs=4))
        atp = ctx.enter_context(tc.tile_pool(name="atp", bufs=3))
        ps_s = ctx.enter_context(tc.tile_pool(name="ps_s", bufs=2, space="PSUM"))
        ps_po = ctx.enter_context(tc.tile_pool(name="ps_po", bufs=2, space="PSUM"))
        ps_pp = ctx.enter_context(tc.tile_pool(name="ps_pp", bufs=2, space="PSUM"))
        dram = ctx.enter_context(tc.tile_pool(name="dram", bufs=1, space="DRAM"))

        # tiny dummy exchange issued at kernel start: absorbs the one-time
        # first-collective setup (~40us) concurrently with the initial
        # DMA/projection phase instead of exposing it on batch 0's critical
        # path
        warm_i = dram.tile([N_CORES, 16], BF16, tag="warm_i", name="warm_i")
        warm_o = dram.tile([N_CORES, 16], BF16, tag="warm_o", name="warm_o")
        # half-batch exchange pieces: half h2 of batch b carries q-cols
        # [1024*h2, 1024*h2+1024); dest core c owns q-cols 1024*h2+128c..+128.
        # in rows = (dest c)*128 + (head h)*64 + dk-row r; cols = local q.
        # Exchange k merges (k-1, h1) with (k, h0) into ONE AllToAll — both
        # pieces' data are ready at t=1 of batch k, and halving the
        # collective count stops back-to-back exchanges from serializing on
        # the CC engine at batch boundaries.
        def _xsize(k):
            return 128 * ((0 < k) + (k < B))

        xin = [dram.tile([N_CORES * 128, _xsize(k)], BF16, tag=f"xin_{k}",
                         name=f"xin_{k}") for k in range(B + 1)]
        xout = [dram.tile([N_CORES * 128, _xsize(k)], BF16, tag=f"xout_{k}",
                          name=f"xout_{k}") for k in range(B + 1)]

        def _piece(b, h2):
            """(exchange index, column offset) of piece (b, h2)."""
            if h2 == 0:
                return b, (128 if b > 0 else 0)
            return b + 1, 0

        # the final piece (3, h2=1) is exchanged as TWO per-t quarters so
        # the t=2 quarter's collective overlaps attention(3); dest core c
        # owns q-cols {1024+512*(t-2)+64c .. +64} of batch 3 for t=2,3.
        # rows = (dest c)*64? no: (c)*128 + 64h + r as usual, cols = 64.
        xq = [dram.tile([N_CORES * 128, 64], BF16, tag=f"xq_{i}",
                        name=f"xq_{i}") for i in range(2)]
        xqo = [dram.tile([N_CORES * 128, 64], BF16, tag=f"xqo_{i}",
                         name=f"xqo_{i}") for i in range(2)]
        # DRAM bounce for the reciprocal-denominator partition broadcast
        rcpd = dram.tile([1, B * NT * 2048], F32, tag="rcpd", name="rcpd")

        # --- constants ---
        triu = consts.tile([128, 128], BF16)
        make_upper_triangular(nc, triu, 1.0, diag=True)
        ident_bf = consts.tile([128, 128], BF16)
        make_identity(nc, ident_bf)
        ones_col = consts.tile([1, 64], BF16)
        nc.vector.memset(ones_col, 1.0)
        # biases + warmup seed ride the ACT queue: keeps the sync queue's
        # head free for the weight/xT loads that gate the first matmul
        bq_sb = consts.tile([HC, 1], F32)
        nc.scalar.dma_start(out=bq_sb, in_=bq[:, :])
        bk_sb = consts.tile([HC, 1], F32)
        nc.scalar.dma_start(out=bk_sb, in_=bk[:, :])
        bv_sb = consts.tile([HC, 1], F32)
        nc.scalar.dma_start(out=bv_sb, in_=bv[:, :])
        bo_sb = consts.tile([128, D], F32)
        nc.scalar.dma_start(out=bo_sb, in_=_bcast(bo, 128, D))

        # --- wk first on sync: proj(0) computes kt first, so only wk + the
        # first 1MB xT quarter gate the first matmul; wq/wv ride the ACT
        # queue so they neither trigger nor transfer ahead of xT on sync ---
        wk_sb = wpool.tile([128, DC, HC], BF16, tag="wk_sb")
        nc.sync.dma_start(out=wk_sb, in_=wk.rearrange("p (c m) -> p c m", c=DC))
        wq_sb = wpool.tile([128, DC, HC], BF16, tag="wq_sb")
        nc.scalar.dma_start(out=wq_sb, in_=wq.rearrange("p (c m) -> p c m", c=DC))
        wv_sb = wpool.tile([128, DC, HC], BF16, tag="wv_sb")
        nc.scalar.dma_start(out=wv_sb, in_=wv.rearrange("p (c m) -> p c m", c=DC))

        # xt is host-blocked so each half-batch load is 128 contiguous
        # 16KB lines: descriptor generation is ~0.9us instead of ~5us
        xt_r = xt.rearrange("p (b u c q) -> p b u c q", b=B, u=2, c=DC)

        def emit_xt_dma(b):
            """Four quarter loads (per s2-half, per c-half) so proj's first
            c-chunks start as soon as the first 1MB lands."""
            parts = []
            for u in range(2):
                tl = []
                for ch in range(2):
                    xh = xtp.tile([128, 4, 1024], BF16, tag=f"xT{u}{ch}",
                                  name=f"xT_{b}_{u}_{ch}")
                    nc.sync.dma_start(
                        out=xh, in_=xt_r[:, b, u, 4 * ch:4 * ch + 4, :])
                    tl.append(xh)
                parts.append(tl)

            def x_at(s2, c):
                return parts[s2][c // 4][:, c % 4, :]

            return x_at

        def proj_setup(b):
            """Allocate batch b's K/Q/V tiles and publish them; the tile
            framework orders attention's reads after the actual writes."""
            kt = kqv.tile([HC, S], BF16, tag="kt", name=f"kt_{b}")
            qt_ = kqv.tile([HC, S], BF16, tag="qt", name=f"qt_{b}")
            vt = kqv.tile([HC, S], BF16, tag="vt", name=f"vt_{b}")
            vsb = kqv.tile([128, ST, H_LOC * 65], BF16, tag="vsb",
                           name=f"vsb_{b}")
            v4 = vsb.rearrange("p s (h o) -> p s h o", o=65)
            nc.vector.memset(v4[:, :, :, 64:65], 1.0)
            kqv_tiles[b] = (kt, qt_, vsb)
            return kt, qt_, vt, v4

        def proj_steps(b, x_at, tiles, s2s, vts):
            """Generator: K^T/Q^T/V^T projection + V PE-transpose for the
            given s2-halves / V-strips of batch b, yielded in PE-dense steps
            so attention emission can interleave them. Phase A (s2=0, strips
            0-7) is all that attention t=0/t=1 needs, so attention(0) starts
            half-way through proj(0)."""
            kt, qt_, vt, v4 = tiles
            for s2 in s2s:
                for dst, w_sb, b_sb in ((kt, wk_sb, bk_sb), (qt_, wq_sb, bq_sb),
                                        (vt, wv_sb, bv_sb)):
                    # one weight load per c serves both 512-chunks of the pair;
                    # yield every 2 c's (~4 MMs) so the interleaved attention
                    # strips never starve behind a long projection burst
                    pp = [ps_pp.tile([128, 512], F32, tag="pp",
                                     name=f"pp_{b}_{s2}_{u}")
                          for u in range(2)]
                    for c in range(DC):
                        for u in range(2):
                            nc.tensor.matmul(
                                pp[u], lhsT=w_sb[:, c, :],
                                rhs=x_at(s2, c)[:, u * 512:(u + 1) * 512],
                                start=(c == 0), stop=(c == DC - 1))
                        if c % 2 == 1:
                            yield None
                    for u in range(2):
                        s4 = 2 * s2 + u
                        nc.vector.tensor_scalar_add(
                            dst[:, s4 * 512:(s4 + 1) * 512], pp[u], b_sb)
                    yield None
            # V natural via PE transposes, DVE-copied into the per-head
            # [V_h|1] layout
            for st_ in vts:
                pt = ps_pp.tile([128, 512], F32, tag="pp",
                                name=f"pt_{b}_{st_}")[:, 0:64].bitcast(BF16)
                nc.tensor.transpose(pt,
                                    vt[:, st_ * 128:(st_ + 1) * 128], ident_bf)
                nc.vector.tensor_copy(
                    v4[:, st_, :, 0:64],
                    pt.rearrange("p (h o) -> p h o", o=64))
                if st_ % 4 == 3:
                    yield None
            yield None

        def oproj_steps(b, h2):
            """Generator: output projection for half h2 of batch b (after
            its half-batch AllToAll). The aT load rides the GpSimd queue,
            which already serializes behind the collective, so it never
            blocks the sync queue."""
            if b == B - 1 and h2 == 1:
                # t=2 quarter was pre-loaded under attention; only the t=3
                # quarter (the one tail-exposed exchange) loads here
                aT = pend_at[0]
                nc.gpsimd.dma_start(
                    out=aT[:, :, 64:128],
                    in_=xqo[1].rearrange("(c p) q -> p c q", p=128))
            else:
                aT = atp.tile([128, DC, 128], BF16, tag="aT",
                              name=f"aT_{b}_{h2}")
                xk, xoff = _piece(b, h2)
                oh = xout[xk].rearrange("(c p) q -> p c q", p=128)
                nc.gpsimd.dma_start(out=aT[:, 0:4, :],
                                    in_=oh[:, 0:4, xoff:xoff + 128])
                nc.gpsimd.dma_start(out=aT[:, 4:8, :],
                                    in_=oh[:, 4:8, xoff:xoff + 128])
            yield None
            pp = [ps_pp.tile([128, 512], F32, tag="pp",
                             name=f"ppo_{b}_{h2}_{nh}")
                  for nh in range(2)]
            for c in range(DC):
                for nh in range(2):
                    nc.tensor.matmul(
                        pp[nh], lhsT=aT[:, c, :],
                        rhs=wo_sb[:, c, nh * 512:(nh + 1) * 512],
                        start=(c == 0), stop=(c == DC - 1))
                if c % 2 == 1:
                    yield None
            # the last batch's stores ride the (then-idle) ACT queue so the
            # final exchange's input writes aren't stuck behind them on sync
            sq = nc.scalar if b == B - 1 else nc.sync
            for nh in range(2):
                osb = opool.tile([128, 512], BF16, tag="osb")
                nc.vector.tensor_add(osb, pp[nh],
                                     bo_sb[:, nh * 512:(nh + 1) * 512])
                sq.dma_start(
                    out=out[b * 256 + h2 * 128:b * 256 + h2 * 128 + 128,
                            nh * 512:(nh + 1) * 512],
                    in_=osb)
            yield None

        def _skip(n):
            for _ in range(n):
                yield None

        def _paced(gen, credit_per_yield):
            """Wrap a generator so each next() only advances it
            `credit_per_yield` steps on average — spreads interleaved work
            evenly across the attention strips instead of front-loading."""
            credit = 0.0
            while True:
                credit += credit_per_yield
                while credit >= 1.0:
                    credit -= 1.0
                    try:
                        next(gen)
                    except StopIteration:
                        return
                yield None

        def emit_coll(k):
            nc.gpsimd.collective_compute(
                "AllToAll", mybir.AluOpType.bypass,
                replica_groups=[list(range(N_CORES))],
                ins=[xin[k].opt()], outs=[xout[k].opt()])

        def emit_attention(b, interleave, tail=None):
            cur_t = 0

            def step():
                next(interleave, None)
                # tail (= oproj of this batch's first half) may only be
                # emitted once its half-collective exists (end of t=1)
                if tail is not None and cur_t >= NT - 1:
                    next(tail, None)

            kt, qt_, vsb = kqv_tiles[b]
            for t in range(NT):
                cur_t = t
                q0 = 512 * t
                nj = 4 * t + 4
                po = [ps_po.tile([65, 512], F32, tag="po",
                                 name=f"po_{b}_{t}_{h}") for h in range(2)]

                def emit_pv(pend, po=po, nj=nj):
                    es_p, jp, relp = pend
                    for h in range(H_LOC):
                        nc.tensor.matmul(
                            po[h][:, relp:512],
                            lhsT=vsb[:, jp, h * 65:(h + 1) * 65],
                            rhs=es_p[:, h * 512 + relp:h * 512 + 512],
                            start=(jp == 0), stop=(jp == nj - 1),
                            skip_group_check=True)

                pend = []
                for j in range(nj):
                    rel = max(0, 128 * j - q0)
                    ps = ps_s.tile([128, 1024], F32, tag="ps")
                    # both heads' scores concurrently via PE row-group tiles
                    nc.tensor.matmul(ps[:, rel:512],
                                     lhsT=kt[0:64, j * 128:(j + 1) * 128],
                                     rhs=qt_[0:64, q0 + rel:q0 + 512],
                                     start=True, stop=True)
                    nc.tensor.matmul(ps[:, 512 + rel:1024],
                                     lhsT=kt[64:128, j * 128:(j + 1) * 128],
                                     rhs=qt_[64:128, q0 + rel:q0 + 512],
                                     start=True, stop=True,
                                     skip_group_check=True)
                    es = epool.tile([128, 1024], BF16, tag="et")
                    # one wide exp covers both heads ([512:512+rel) is unused
                    # garbage on diagonal strips, never consumed by PV)
                    nc.scalar.activation(es[:, rel:1024], ps[:, rel:1024],
                                         mybir.ActivationFunctionType.Exp,
                                         scale=1.0 / np.sqrt(DK))
                    if 128 * j >= q0:  # diagonal strip: causal 0/1 mask
                        nc.vector.tensor_mul(es[:, rel:rel + 128],
                                             es[:, rel:rel + 128], triu)
                        nc.vector.tensor_mul(es[:, 512 + rel:512 + rel + 128],
                                             es[:, 512 + rel:512 + rel + 128],
                                             triu)
                    # PV trails the scores by TWO strips so each exp has a
                    # full extra strip of slack before the in-order PE queue
                    # reaches its consumer
                    if len(pend) == 2:
                        emit_pv(pend.pop(0))
                        step()
                    pend.append((es, j, rel))
                    step()
                for p_ in pend:
                    emit_pv(p_)
                    step()

                # normalize (no transposes: po is already [head-dim, q]).
                # DVE copy to SBUF releases the po PSUM banks immediately.
                # The denominator row lives on ONE partition, where a DVE
                # reciprocal would be serial (8 cyc/elem = 3.4us); instead
                # DMA-spread it across 64 lanes, reciprocal there (0.1us),
                # gather back and broadcast-read. The whole chain rides the
                # GpSimd DMA queue so it never blocks sync-queue traffic.
                # The whole chain rides the SYNC queue so the GpSimd
                # queue carries only collectives + aT loads: half-batch
                # exchanges then trigger as soon as their data semaphores
                # fire instead of queuing behind the previous collective
                dq = nc.sync
                last = (b == B - 1 and t == NT - 1)
                poc = rpool.tile([65, 1024], F32, tag="poc")
                nc.vector.tensor_copy(poc[:, 0:512], po[0])
                nc.vector.tensor_copy(poc[:, 512:1024], po[1])
                slot = (4 * b + t) * 2048
                if last:
                    # final chunk: 1/denom = exp(-ln(denom)) on ScalarE (Ln
                    # and Exp share an activation table). One DRAM roundtrip
                    # instead of two shortens the fully-exposed tail chain;
                    # everywhere else this would head-of-line block the next
                    # chunk's exps in the strict-FIFO ACT queue, so the
                    # steady-state chunks keep the DVE spread below.
                    lnr = rcpool.tile([1, 1024], F32, tag="lnr")
                    nc.scalar.activation(lnr[:, 0:512], po[0][64:65, :],
                                         mybir.ActivationFunctionType.Ln)
                    nc.scalar.activation(lnr[:, 512:1024], po[1][64:65, :],
                                         mybir.ActivationFunctionType.Ln)
                    rcw = rcpool.tile([1, 1024], BF16, tag="rcw")
                    nc.scalar.activation(rcw, lnr,
                                         mybir.ActivationFunctionType.Exp,
                                         scale=-1.0)
                    # broadcast across the 64 dk lanes via a K=1 matmul with
                    # a ones column — the PE is idle at the tail, and this
                    # skips the DRAM roundtrip latency entirely
                    rb = [ps_pp.tile([64, 512], F32, tag="pp",
                                     name=f"rbp_{hh}") for hh in range(2)]
                    for hh in range(2):
                        nc.tensor.matmul(
                            rb[hh], lhsT=ones_col,
                            rhs=rcw[:, hh * 512:(hh + 1) * 512],
                            start=True, stop=True)
                else:
                    rb = bpool.tile([64, 1024], F32, tag="rb")
                    # denominator row lives on ONE partition where DVE
                    # reciprocal would be serial: DMA-spread it across 64
                    # lanes, reciprocal there, gather back + broadcast-read
                    dq.dma_start(out=rcpd[0:1, slot:slot + 1024],
                                 in_=poc[64:65, :])
                    dsc = spool.tile([64, 16], F32, tag="dsc")
                    dq.dma_start(
                        out=dsc,
                        in_=rcpd[0:1, slot:slot + 1024].rearrange(
                            "a (b c) -> (a b) c", c=16))
                    rcv = spool.tile([64, 16], F32, tag="rcv")
                    nc.vector.reciprocal(rcv, dsc)
                    dq.dma_start(
                        out=rcpd[0:1, slot + 1024:slot + 2048].rearrange(
                            "a (b c) -> (a b) c", c=16),
                        in_=rcv)
                    dq.dma_start(
                        out=rb,
                        in_=rcpd[0:1, slot + 1024:slot + 2048].to_broadcast(
                            [64, 1024]))
                # normalized output straight into the half-batch exchange
                # buffer: dest core c owns local q-cols 128c..128c+128 of
                # this half; one DMA per head covers the 4 dest chunks this
                # t contributes to (rows 128c+64h+r, cols 128cc+qq)
                if b == B - 1 and t >= 2:
                    ih4 = xq[t - 2].rearrange("(c h r) q -> h r c q",
                                              h=2, r=64)
                    for h in range(H_LOC):
                        st = spool.tile([64, 512], BF16, tag="st")
                        rbs = rb[h] if isinstance(rb, list) else \
                            rb[0:64, h * 512:(h + 1) * 512]
                        nc.vector.tensor_mul(
                            st, poc[0:64, h * 512:(h + 1) * 512], rbs)
                        dq.dma_start(
                            out=ih4[h, :, :, :],
                            in_=st.rearrange("p (c q) -> p c q", q=64))
                else:
                    xk, xoff = _piece(b, t // 2)
                    ih4 = xin[xk].rearrange("(c h r) q -> h r c q", h=2, r=64)
                    c0 = 4 * (t % 2)
                    for h in range(H_LOC):
                        st = spool.tile([64, 512], BF16, tag="st")
                        rbs = rb[h] if isinstance(rb, list) else \
                            rb[0:64, h * 512:(h + 1) * 512]
                        nc.vector.tensor_mul(
                            st, poc[0:64, h * 512:(h + 1) * 512], rbs)
                        dq.dma_start(
                            out=ih4[h, :, c0:c0 + 4, xoff:xoff + 128],
                            in_=st.rearrange("p (c q) -> p c q", q=128))
                step()

                if t == 1:
                    # exchange b = (b-1, h1) + (b, h0): fires mid-attention
                    emit_coll(b)
                elif t >= 2 and b == B - 1:
                    # final-piece quarters: t=2's exchange overlaps t=3's
                    # compute; only t=3's 128KB quarter is tail-exposed
                    nc.gpsimd.collective_compute(
                        "AllToAll", mybir.AluOpType.bypass,
                        replica_groups=[list(range(N_CORES))],
                        ins=[xq[t - 2].opt()], outs=[xqo[t - 2].opt()])
                    if t == 2:
                        # pre-load the tail oproj's first aT half under t=3
                        a31 = atp.tile([128, DC, 128], BF16, tag="aT",
                                       name="aT_tail")
                        nc.gpsimd.dma_start(
                            out=a31[:, :, 0:64],
                            in_=xqo[0].rearrange("(c p) q -> p c q", p=128))
                        pend_at[0] = a31

        # warm up the collective path while the pipeline head loads/projects
        warm_sb = consts.tile([N_CORES, 16], BF16)
        nc.scalar.dma_start(out=warm_sb, in_=xt[0:N_CORES, 0:16])
        nc.gpsimd.dma_start(out=warm_i[:, :], in_=warm_sb)
        nc.gpsimd.collective_compute(
            "AllToAll", mybir.AluOpType.bypass,
            replica_groups=[list(range(N_CORES))],
            ins=[warm_i.opt()], outs=[warm_o.opt()])

        kqv_tiles = {}
        pend_at = [None]
        t0 = proj_setup(0)
        xT0 = emit_xt_dma(0)
        # wo is only needed by oproj(0), well after proj(0): load it after
        # xT(0) so it doesn't delay the pipeline head on the sync DMA queue
        wo_sb = wpool.tile([128, DC, D], BF16, tag="wo_sb")
        nc.sync.dma_start(out=wo_sb, in_=wo.rearrange("p (c m) -> p c m", c=DC))

        # phase A only: attention(0)'s t=0/t=1 need just the first halves
        for _ in proj_steps(0, xT0, t0, [0], range(8)):
            pass
        for b in range(B):
            gens = []
            tail_gen = None
            if b == 0:
                # proj(0) phase B interleaves into attention(0)'s early
                # strips (done well before t=2 needs it)
                gens.append(proj_steps(0, xT0, t0, [1], range(8, 16)))
            if b + 1 < B:
                xTn = emit_xt_dma(b + 1)
                tn = proj_setup(b + 1)
                gens.append(proj_steps(b + 1, xTn, tn, [0, 1], range(16)))
            if b >= 1:
                # oproj halves land mid/late-batch: keeps PE work spread so
                # the HAM clock never sees a >3.4us idle window; half 0's
                # exchange completed during attention(b-1), half 1's
                # completes ~60% into this batch
                gens.append(_skip(6 if b == B - 1 else 0))
                gens.append(oproj_steps(b - 1, 0))
                gens.append(_skip(12))
                gens.append(oproj_steps(b - 1, 1))
            if b == B - 1:
                # first half of the last batch's oproj interleaves into the
                # final q-chunk's strips (its collective fired at t=1)
                tail_gen = _paced(oproj_steps(b, 0), 0.6)
            credit = 1.1 if b == 0 else (0.75 if b < B - 1 else 0.3)
            inter = _paced(chain(*gens), credit) if gens else iter(())
            emit_attention(b, inter, tail_gen)
            for _ in inter:
                pass
            if tail_gen is not None:
                for _ in tail_gen:
                    pass
        # only the last 256KB exchange + 16 matmuls are exposed at the tail
        for _ in oproj_steps(B - 1, 1):
            pass

    nc.compile()
    return nc


_NC_CACHE = None


def _get_program():
    global _NC_CACHE
    if _NC_CACHE is None:
        _NC_CACHE = build_program()
    return _NC_CACHE


def _blk(w):
    """[D, M] -> [128, DC*M]: partition-major c-chunk blocking so the SBUF
    load is 128 contiguous lines."""
    m = w.shape[1]
    return np.ascontiguousarray(
        w.reshape(DC, 128, m).transpose(1, 0, 2).reshape(128, DC * m))


def _make_in_maps(x, w_qkv, b_qkv, w_o, b_o):
    x = np.asarray(x, dtype=np.float32).reshape(B * S, D)
    # [128, (b u c q)]: per-partition contiguous 16KB lines per half-batch
    xt = np.ascontiguousarray(
        x.T.reshape(DC, 128, B, 2, 1024).transpose(1, 2, 3, 0, 4).reshape(
            128, B * 2 * DC * 1024)).astype(BF16_NP)
    w_qkv = np.asarray(w_qkv, dtype=np.float32)
    b_qkv = np.asarray(b_qkv, dtype=np.float32)
    wo_bf = _blk(np.asarray(w_o, dtype=np.float32)).astype(BF16_NP)
    b_o = np.asarray(b_o, dtype=np.float32).reshape(1, D)
    in_maps = []
    for c in range(N_CORES):
        lo = c * HC
        hi = lo + HC
        in_maps.append({
            "xt": xt,
            "wq": _blk(w_qkv[:, lo:hi]).astype(BF16_NP),
            "wk": _blk(w_qkv[:, D + lo:D + hi]).astype(BF16_NP),
            "wv": _blk(w_qkv[:, 2 * D + lo:2 * D + hi]).astype(BF16_NP),
            "bq": np.ascontiguousarray(b_qkv[lo:hi].reshape(HC, 1)),
            "bk": np.ascontiguousarray(b_qkv[D + lo:D + hi].reshape(HC, 1)),
            "bv": np.ascontiguousarray(b_qkv[2 * D + lo:2 * D + hi].reshape(HC, 1)),
            "wo": wo_bf,
            "bo": b_o,
        })
    return in_maps


def _assemble(results):
    out = np.empty((B, S, D), dtype=np.float32)
    for c in range(N_CORES):
        for b in range(B):
            for h2 in range(2):
                r0 = b * 256 + 128 * h2
                blk = np.asarray(results[c]["out"][r0:r0 + 128],
                                 dtype=np.float32)
                if b == B - 1 and h2 == 1:
                    # final piece is exchanged per-t: core c owns 64 q-rows
                    # of each of t=2 and t=3
                    out[b, 1024 + 64 * c:1024 + 64 * c + 64, :] = blk[0:64]
                    out[b, 1536 + 64 * c:1536 + 64 * c + 64, :] = blk[64:128]
                else:
                    q0 = 1024 * h2 + 128 * c
                    out[b, q0:q0 + 128, :] = blk
    return out


def run(x, mask, w_qkv, b_qkv, w_o, b_o, trace=False, **trace_kwargs):
    """Run on hardware; returns (output, BassKernelResults)."""
    nc = _get_program()
    in_maps = _make_in_maps(x, w_qkv, b_qkv, w_o, b_o)
    res = run_bass_kernel_spmd(nc, in_maps, list(range(N_CORES)),
                               trace=trace, **trace_kwargs)
    return _assemble(res.results), res


def kernel(x, mask, w_qkv, b_qkv, w_o, b_o):
    out, _ = run(x, mask, w_qkv, b_qkv, w_o, b_o)
    return out


# revision 28
# speedup vs baseline: 1.0069x; 1.0069x over previous
"""MultiHeadAttention (B=4, S=2048, D=1024, H=16, causal) on 8 TRN2 NeuronCores.

Sharding: tensor-parallel over heads across all 8 cores (2 heads/core, all 4
batches processed locally; identical SPMD control flow on every core). After
attention, per-batch the attention outputs are redistributed by TWO
half-batch AllToAlls (q-columns [0:1024) and [1024:2048)); each core then
runs the output projection for its 128 q-rows of each half. The first half's
collective fires at ~50% of the batch's attention, so the output projection
never head-of-line blocks the in-order PE queue, and the exposed tail after
the last batch is just one 256KB collective + 16 matmuls.

Per-core pipeline (all matmuls bf16 with f32 PSUM accumulation):
  - x arrives host-transposed as x^T [D, B*S] bf16, loaded in two
    single-trigger half-chunks per batch so projection starts ~9us in;
    K^T/Q^T/V^T via w-stationary matmuls, bias added on the DVE eviction.
    V^T is transposed to natural V by PE transposes with a ones column per
    head so the PV matmul also produces the softmax denominator.
  - Scores are computed transposed ([k, q] = K @ Q^T) in 512-wide q-chunks.
    The two heads' score matmuls are issued as row-group tiles (K=64 each,
    rows 0-63 / 64-127) so they can execute concurrently in the PE array and
    land in the two PSUM banks of one [128, 1024] tile; a single wide exp on
    ScalarE covers both heads (1/sqrt(dk) scale folded in; no max subtraction
    needed: |scores| <~ 2.6). Causal mask = 0/1 triangular multiply on
    diagonal tiles only; fully-masked tiles are never computed.
  - PV: [V_h|1] stationary, exp chunks stream, accumulating po_h = [out^T;
    denom] [65, 512] in PSUM, one k-strip behind scores/exp.
  - po is already in the [head-dim, q] layout the AllToAll needs: the
    denominator row is reciprocal'd via a DRAM bounce that spreads it over
    64 lanes, and a single DVE multiply writes the normalized bf16 output
    straight into the half-batch exchange buffer (one DMA per (t, head)).
  - Output projection halves are interleaved into the next batch's
    attention; their aT loads ride the GpSimd queue (already serialized
    behind the collectives) so they never block the sync queue.
"""

import sys

if "/opt/trn_rl_repo" not in sys.path:
    sys.path.insert(0, "/opt/trn_rl_repo")

from contextlib import ExitStack
from itertools import chain

import ml_dtypes
import numpy as np

import concourse.bacc as bacc
import concourse.bass as bass
import concourse.mybir as mybir
import concourse.tile as tile
from concourse.bass_utils import run_bass_kernel_spmd
from concourse.masks import make_identity, make_upper_triangular

N_CORES = 8
B = 4
S = 2048
D = 1024
H_TOT = 16
DK = 64
H_LOC = H_TOT // N_CORES  # 2 heads per core
HC = H_LOC * DK  # 128 head-cols per core
ST = S // 128  # 16 k-strips per batch
DC = D // 128  # 8 d_model chunks
NT = S // 512  # 4 q-chunks per batch
BQ = (B * S) // N_CORES  # 1024 (batch,seq) rows per core after AllToAll

F32 = mybir.dt.float32
BF16 = mybir.dt.bfloat16
BF16_NP = ml_dtypes.bfloat16


def _bcast(handle, rows, cols):
    """AP reading a [1, cols] DRAM tensor broadcast over `rows` partitions."""
    return bass.AP(tensor=handle, offset=0, ap=[[0, rows], [1, cols]])


def build_program():
    nc = bacc.Bacc("TRN2", target_bir_lowering=False, debug=False,
                   num_devices=N_CORES)

    xt = nc.declare_dram_parameter("xt", [128, B * 2 * DC * 1024], BF16,
                               isOutput=False)
    wq = nc.declare_dram_parameter("wq", [128, DC * HC], BF16, isOutput=False)
    wk = nc.declare_dram_parameter("wk", [128, DC * HC], BF16, isOutput=False)
    wv = nc.declare_dram_parameter("wv", [128, DC * HC], BF16, isOutput=False)
    bq = nc.declare_dram_parameter("bq", [HC, 1], F32, isOutput=False)
    bk = nc.declare_dram_parameter("bk", [HC, 1], F32, isOutput=False)
    bv = nc.declare_dram_parameter("bv", [HC, 1], F32, isOutput=False)
    wo = nc.declare_dram_parameter("wo", [128, DC * D], BF16, isOutput=False)
    bo = nc.declare_dram_parameter("bo", [1, D], F32, isOutput=False)
    out = nc.declare_dram_parameter("out", [BQ, D], BF16, isOutput=True)

    with ExitStack() as ctx:
        tc = ctx.enter_context(tile.TileContext(nc))

        consts = ctx.enter_context(tc.tile_pool(name="consts", bufs=1))
        wpool = ctx.enter_context(tc.tile_pool(name="wpool", bufs=1))
        xtp = ctx.enter_context(tc.tile_pool(name="xtp", bufs=2))
        kqv = ctx.enter_context(tc.tile_pool(name="kqv", bufs=2))
        # 5 slots: with PV trailing two strips, es(j)'s allocation must not
        # wait on pv(j-4) execution lag (4 slots is exactly tight)
        epool = ctx.enter_context(tc.tile_pool(name="epool", bufs=5))
        # normalize-chain pools are deep enough (4+ chunks = one full batch)
        # to ride out collective-induced GpSimd-queue blocking without
        # backpressuring the PV/exp pipeline
        rpool = ctx.enter_context(tc.tile_pool(name="rpool", bufs=5))
        bpool = ctx.enter_context(tc.tile_pool(name="bpool", bufs=5))
        spool = ctx.enter_context(tc.tile_pool(name="spool", bufs=10))
        rcpool = ctx.enter_context(tc.tile_pool(name="rcpool", bufs=2))
        # 4 slots: at the batch-3 tail two oproj halves run back-to-back (4
        # osb tiles) while their store DMAs queue behind the final normalize
        # chain on sync — 3 slots would stall the 4th DVE add
        opool = ctx.enter_context(tc.tile_pool(name="opool", bufs=4))
        atp = ctx.enter_context(tc.tile_pool(name="atp", bufs=3))
        ps_s = ctx.enter_context(tc.tile_pool(name="ps_s", bufs=2, space="PSUM"))
        ps_po = ctx.enter_context(tc.tile_pool(name="ps_po", bufs=2, space="PSUM"))
        ps_pp = ctx.enter_context(tc.tile_pool(name="ps_pp", bufs=2, space="PSUM"))
        dram = ctx.enter_context(tc.tile_pool(name="dram", bufs=1, space="DRAM"))

        # tiny dummy exchange issued at kernel start: absorbs the one-time
        # first-collective setup (~40us) concurrently with the initial
        # DMA/projection phase instead of exposing it on batch 0's critical
        # path
        warm_i = dram.tile([N_CORES, 16], BF16, tag="warm_i", name="warm_i")
        warm_o = dram.tile([N_CORES, 16], BF16, tag="warm_o", name="warm_o")
        # half-batch exchange pieces: half h2 of batch b carries q-cols
        # [1024*h2, 1024*h2+1024); dest core c owns q-cols 1024*h2+128c..+128.
        # in rows = (dest c)*128 + (head h)*64 + dk-row r; cols = local q.
        # Exchange k merges (k-1, h1) with (k, h0) into ONE AllToAll — both
        # pieces' data are ready at t=1 of batch k, and halving the
        # collective count stops back-to-back exchanges from serializing on
        # the CC engine at batch boundaries.
        def _xsize(k):
            return 128 * ((0 < k) + (k < B))

        xin = [dram.tile([N_CORES * 128, _xsize(k)], BF16, tag=f"xin_{k}",
                         name=f"xin_{k}") for k in range(B + 1)]
        xout = [dram.tile([N_CORES * 128, _xsize(k)], BF16, tag=f"xout_{k}",
                          name=f"xout_{k}") for k in range(B + 1)]

        def _piece(b, h2):
            """(exchange index, column offset) of piece (b, h2)."""
            if h2 == 0:
                return b, (128 if b > 0 else 0)
            return b + 1, 0

        # the final piece (3, h2=1) is exchanged as TWO per-t quarters so
        # the t=2 quarter's collective overlaps attention(3); dest core c
        # owns q-cols {1024+512*(t-2)+64c .. +64} of batch 3 for t=2,3.
        # rows = (dest c)*64? no: (c)*128 + 64h + r as usual, cols = 64.
        xq = [dram.tile([N_CORES * 128, 64], BF16, tag=f"xq_{i}",
                        name=f"xq_{i}") for i in range(2)]
        xqo = [dram.tile([N_CORES * 128, 64], BF16, tag=f"xqo_{i}",
                         name=f"xqo_{i}") for i in range(2)]
        # DRAM bounce for the reciprocal-denominator partition broadcast
        rcpd = dram.tile([1, B * NT * 2048], F32, tag="rcpd", name="rcpd")

        # --- constants ---
        triu = consts.tile([128, 128], BF16)
        make_upper_triangular(nc, triu, 1.0, diag=True)
        ident_bf = consts.tile([128, 128], BF16)
        make_identity(nc, ident_bf)
        ones_col = consts.tile([1, 64], BF16)
        nc.vector.memset(ones_col, 1.0)
        # biases + warmup seed ride the ACT queue: keeps the sync queue's
        # head free for the weight/xT loads that gate the first matmul
        bq_sb = consts.tile([HC, 1], F32)
        nc.scalar.dma_start(out=bq_sb, in_=bq[:, :])
        bk_sb = consts.tile([HC, 1], F32)
        nc.scalar.dma_start(out=bk_sb, in_=bk[:, :])
        bv_sb = consts.tile([HC, 1], F32)
        nc.scalar.dma_start(out=bv_sb, in_=bv[:, :])
        bo_sb = consts.tile([128, D], F32)
        nc.scalar.dma_start(out=bo_sb, in_=_bcast(bo, 128, D))

        # --- wk first on sync: proj(0) computes kt first, so only wk + the
        # first 1MB xT quarter gate the first matmul; wq/wv ride the ACT
        # queue so they neither trigger nor transfer ahead of xT on sync ---
        wk_sb = wpool.tile([128, DC, HC], BF16, tag="wk_sb")
        nc.sync.dma_start(out=wk_sb, in_=wk.rearrange("p (c m) -> p c m", c=DC))
        wq_sb = wpool.tile([128, DC, HC], BF16, tag="wq_sb")
        nc.scalar.dma_start(out=wq_sb, in_=wq.rearrange("p (c m) -> p c m", c=DC))
        wv_sb = wpool.tile([128, DC, HC], BF16, tag="wv_sb")
        nc.scalar.dma_start(out=wv_sb, in_=wv.rearrange("p (c m) -> p c m", c=DC))

        # xt is host-blocked so each half-batch load is 128 contiguous
        # 16KB lines: descriptor generation is ~0.9us instead of ~5us
        xt_r = xt.rearrange("p (b u c q) -> p b u c q", b=B, u=2, c=DC)

        def emit_xt_dma(b):
            """Four quarter loads (per s2-half, per c-half) so proj's first
            c-chunks start as soon as the first 1MB lands."""
            parts = []
            for u in range(2):
                tl = []
                for ch in range(2):
                    xh = xtp.tile([128, 4, 1024], BF16, tag=f"xT{u}{ch}",
                                  name=f"xT_{b}_{u}_{ch}")
                    nc.sync.dma_start(
                        out=xh, in_=xt_r[:, b, u, 4 * ch:4 * ch + 4, :])
                    tl.append(xh)
                parts.append(tl)

            def x_at(s2, c):
                return parts[s2][c // 4][:, c % 4, :]

            return x_at

        def proj_setup(b):
            """Allocate batch b's K/Q/V tiles and publish them; the tile
            framework orders attention's reads after the actual writes."""
            kt = kqv.tile([HC, S], BF16, tag="kt", name=f"kt_{b}")
            qt_ = kqv.tile([HC, S], BF16, tag="qt", name=f"qt_{b}")
            vt = kqv.tile([HC, S], BF16, tag="vt", name=f"vt_{b}")
            vsb = kqv.tile([128, ST, H_LOC * 65], BF16, tag="vsb",
                           name=f"vsb_{b}")
            v4 = vsb.rearrange("p s (h o) -> p s h o", o=65)
            nc.vector.memset(v4[:, :, :, 64:65], 1.0)
            kqv_tiles[b] = (kt, qt_, vsb)
            return kt, qt_, vt, v4

        def proj_steps(b, x_at, tiles, s2s, vts):
            """Generator: K^T/Q^T/V^T projection + V PE-transpose for the
            given s2-halves / V-strips of batch b, yielded in PE-dense steps
            so attention emission can interleave them. Phase A (s2=0, strips
            0-7) is all that attention t=0/t=1 needs, so attention(0) starts
            half-way through proj(0)."""
            kt, qt_, vt, v4 = tiles
            for s2 in s2s:
                for dst, w_sb, b_sb in ((kt, wk_sb, bk_sb), (qt_, wq_sb, bq_sb),
                                        (vt, wv_sb, bv_sb)):
                    # one weight load per c serves both 512-chunks of the pair;
                    # yield every 2 c's (~4 MMs) so the interleaved attention
                    # strips never starve behind a long projection burst
                    pp = [ps_pp.tile([128, 512], F32, tag="pp",
                                     name=f"pp_{b}_{s2}_{u}")
                          for u in range(2)]
                    for c in range(DC):
                        for u in range(2):
                            nc.tensor.matmul(
                                pp[u], lhsT=w_sb[:, c, :],
                                rhs=x_at(s2, c)[:, u * 512:(u + 1) * 512],
                                start=(c == 0), stop=(c == DC - 1))
                        if c % 2 == 1:
                            yield None
                    for u in range(2):
                        s4 = 2 * s2 + u
                        nc.vector.tensor_scalar_add(
                            dst[:, s4 * 512:(s4 + 1) * 512], pp[u], b_sb)
                    yield None
            # V natural via PE transposes, DVE-copied into the per-head
            # [V_h|1] layout
            for st_ in vts:
                pt = ps_pp.tile([128, 512], F32, tag="pp",
                                name=f"pt_{b}_{st_}")[:, 0:64].bitcast(BF16)
                nc.tensor.transpose(pt,
                                    vt[:, st_ * 128:(st_ + 1) * 128], ident_bf)
                nc.vector.tensor_copy(
                    v4[:, st_, :, 0:64],
                    pt.rearrange("p (h o) -> p h o", o=64))
                if st_ % 4 == 3:
                    yield None
            yield None

        def oproj_steps(b, h2):
            """Generator: output projection for half h2 of batch b (after
            its half-batch AllToAll). The aT load rides the GpSimd queue,
            which already serializes behind the collective, so it never
            blocks the sync queue."""
            if b == B - 1 and h2 == 1:
                # t=2 quarter was pre-loaded under attention; only the t=3
                # quarter (the one tail-exposed exchange) loads here
                aT = pend_at[0]
                nc.gpsimd.dma_start(
                    out=aT[:, :, 64:128],
                    in_=xqo[1].rearrange("(c p) q -> p c q", p=128))
            else:
                aT = atp.tile([128, DC, 128], BF16, tag="aT",
                              name=f"aT_{b}_{h2}")
                xk, xoff = _piece(b, h2)
                oh = xout[xk].rearrange("(c p) q -> p c q", p=128)
                nc.gpsimd.dma_start(out=aT[:, 0:4, :],
                                    in_=oh[:, 0:4, xoff:xoff + 128])
                nc.gpsimd.dma_start(out=aT[:, 4:8, :],
                                    in_=oh[:, 4:8, xoff:xoff + 128])
            yield None
            pp = [ps_pp.tile([128, 512], F32, tag="pp",
                             name=f"ppo_{b}_{h2}_{nh}")
                  for nh in range(2)]
            for c in range(DC):
                for nh in range(2):
                    nc.tensor.matmul(
                        pp[nh], lhsT=aT[:, c, :],
                        rhs=wo_sb[:, c, nh * 512:(nh + 1) * 512],
                        start=(c == 0), stop=(c == DC - 1))
                if c % 2 == 1:
                    yield None
            # the last batch's stores ride the (then-idle) ACT queue so the
            # final exchange's input writes aren't stuck behind them on sync
            sq = nc.scalar if b == B - 1 else nc.sync
            for nh in range(2):
                osb = opool.tile([128, 512], BF16, tag="osb")
                nc.vector.tensor_add(osb, pp[nh],
                                     bo_sb[:, nh * 512:(nh + 1) * 512])
                sq.dma_start(
                    out=out[b * 256 + h2 * 128:b * 256 + h2 * 128 + 128,
                            nh * 512:(nh + 1) * 512],
                    in_=osb)
            yield None

        def _skip(n):
            for _ in range(n):
                yield None

        def _paced(gen, credit_per_yield):
            """Wrap a generator so each next() only advances it
            `credit_per_yield` steps on average — spreads interleaved work
            evenly across the attention strips instead of front-loading."""
            credit = 0.0
            while True:
                credit += credit_per_yield
                while credit >= 1.0:
                    credit -= 1.0
                    try:
                        next(gen)
                    except StopIteration:
                        return
                yield None

        def emit_coll(k):
            nc.gpsimd.collective_compute(
                "AllToAll", mybir.AluOpType.bypass,
                replica_groups=[list(range(N_CORES))],
                ins=[xin[k].opt()], outs=[xout[k].opt()])

        def emit_attention(b, interleave, tail=None):
            cur_t = 0

            def step():
                next(interleave, None)
                # tail (= oproj of this batch's first half) may only be
                # emitted once its half-collective exists (end of t=1)
                if tail is not None and cur_t >= NT - 1:
                    next(tail, None)

            kt, qt_, vsb = kqv_tiles[b]
            for t in range(NT):
                cur_t = t
                q0 = 512 * t
                nj = 4 * t + 4
                po = [ps_po.tile([65, 512], F32, tag="po",
                                 name=f"po_{b}_{t}_{h}") for h in range(2)]

                def emit_pv(pend, po=po, nj=nj):
                    es_p, jp, relp = pend
                    for h in range(H_LOC):
                        nc.tensor.matmul(
                            po[h][:, relp:512],
                            lhsT=vsb[:, jp, h * 65:(h + 1) * 65],
                            rhs=es_p[:, h * 512 + relp:h * 512 + 512],
                            start=(jp == 0), stop=(jp == nj - 1),
                            skip_group_check=True)

                pend = []
                for j in range(nj):
                    rel = max(0, 128 * j - q0)
                    ps = ps_s.tile([128, 1024], F32, tag="ps")
                    # both heads' scores concurrently via PE row-group tiles
                    nc.tensor.matmul(ps[:, rel:512],
                                     lhsT=kt[0:64, j * 128:(j + 1) * 128],
                                     rhs=qt_[0:64, q0 + rel:q0 + 512],
                                     start=True, stop=True)
                    nc.tensor.matmul(ps[:, 512 + rel:1024],
                                     lhsT=kt[64:128, j * 128:(j + 1) * 128],
                                     rhs=qt_[64:128, q0 + rel:q0 + 512],
                                     start=True, stop=True,
                                     skip_group_check=True)
                    es = epool.tile([128, 1024], BF16, tag="et")
                    # one wide exp covers both heads ([512:512+rel) is unused
                    # garbage on diagonal strips, never consumed by PV)
                    nc.scalar.activation(es[:, rel:1024], ps[:, rel:1024],
                                         mybir.ActivationFunctionType.Exp,
                                         scale=1.0 / np.sqrt(DK))
                    if 128 * j >= q0:  # diagonal strip: causal 0/1 mask
                        nc.vector.tensor_mul(es[:, rel:rel + 128],
                                             es[:, rel:rel + 128], triu)
                        nc.vector.tensor_mul(es[:, 512 + rel:512 + rel + 128],
                                             es[:, 512 + rel:512 + rel + 128],
                                             triu)
                    # PV trails the scores by TWO strips so each exp has a
                    # full extra strip of slack before the in-order PE queue
                    # reaches its consumer
                    if len(pend) == 2:
                        emit_pv(pend.pop(0))
                        step()
                    pend.append((es, j, rel))
                    step()
                for p_ in pend:
                    emit_pv(p_)
                    step()

                # normalize (no transposes: po is already [head-dim, q]).
                # DVE copy to SBUF releases the po PSUM banks immediately.
                # The denominator row lives on ONE partition, where a DVE
                # reciprocal would be serial (8 cyc/elem = 3.4us); instead
                # DMA-spread it across 64 lanes, reciprocal there (0.1us),
                # gather back and broadcast-read. The whole chain rides the
                # GpSimd DMA queue so it never blocks sync-queue traffic.
                # The whole chain rides the SYNC queue so the GpSimd
                # queue carries only collectives + aT loads: half-batch
                # exchanges then trigger as soon as their data semaphores
                # fire instead of queuing behind the previous collective
                dq = nc.sync
                last = (b == B - 1 and t == NT - 1)
                poc = rpool.tile([65, 1024], F32, tag="poc")
                nc.vector.tensor_copy(poc[:, 0:512], po[0])
                nc.vector.tensor_copy(poc[:, 512:1024], po[1])
                slot = (4 * b + t) * 2048
                if last:
                    # final chunk: 1/denom = exp(-ln(denom)) on ScalarE (Ln
                    # and Exp share an activation table). One DRAM roundtrip
                    # instead of two shortens the fully-exposed tail chain;
                    # everywhere else this would head-of-line block the next
                    # chunk's exps in the strict-FIFO ACT queue, so the
                    # steady-state chunks keep the DVE spread below.
                    lnr = rcpool.tile([1, 1024], F32, tag="lnr")
                    nc.scalar.activation(lnr[:, 0:512], po[0][64:65, :],
                                         mybir.ActivationFunctionType.Ln)
                    nc.scalar.activation(lnr[:, 512:1024], po[1][64:65, :],
                                         mybir.ActivationFunctionType.Ln)
                    rcw = rcpool.tile([1, 1024], BF16, tag="rcw")
                    nc.scalar.activation(rcw, lnr,
                                         mybir.ActivationFunctionType.Exp,
                                         scale=-1.0)
                    # broadcast across the 64 dk lanes via a K=1 matmul with
                    # a ones column — the PE is idle at the tail, and this
                    # skips the DRAM roundtrip latency entirely
                    rb = [ps_pp.tile([64, 512], F32, tag="pp",
                                     name=f"rbp_{hh}") for hh in range(2)]
                    for hh in range(2):
                        nc.tensor.matmul(
                            rb[hh], lhsT=ones_col,
                            rhs=rcw[:, hh * 512:(hh + 1) * 512],
                            start=True, stop=True)
                else:
                    rb = bpool.tile([64, 1024], F32, tag="rb")
                    # denominator row lives on ONE partition where DVE
                    # reciprocal would be serial: DMA-spread it across 64
                    # lanes, reciprocal there, gather back + broadcast-read
                    dq.dma_start(out=rcpd[0:1, slot:slot + 1024],
                                 in_=poc[64:65, :])
                    dsc = spool.tile([64, 16], F32, tag="dsc")
                    dq.dma_start(
                        out=dsc,
                        in_=rcpd[0:1, slot:slot + 1024].rearrange(
                            "a (b c) -> (a b) c", c=16))
                    rcv = spool.tile([64, 16], F32, tag="rcv")
                    nc.vector.reciprocal(rcv, dsc)
                    dq.dma_start(
                        out=rcpd[0:1, slot + 1024:slot + 2048].rearrange(
                            "a (b c) -> (a b) c", c=16),
                        in_=rcv)
                    dq.dma_start(
                        out=rb,
                        in_=rcpd[0:1, slot + 1024:slot + 2048].to_broadcast(
                            [64, 1024]))
                # normalized output straight into the half-batch exchange
                # buffer: dest core c owns local q-cols 128c..128c+128 of
                # this half; one DMA per head covers the 4 dest chunks this
                # t contributes to (rows 128c+64h+r, cols 128cc+qq)
                if b == B - 1 and t >= 2:
                    ih4 = xq[t - 2].rearrange("(c h r) q -> h r c q",
                                              h=2, r=64)
                    for h in range(H_LOC):
                        st = spool.tile([64, 512], BF16, tag="st")
                        rbs = rb[h] if isinstance(rb, list) else \
                            rb[0:64, h * 512:(h + 1) * 512]
                        nc.vector.tensor_mul(
                            st, poc[0:64, h * 512:(h + 1) * 512], rbs)
                        dq.dma_start(
                            out=ih4[h, :, :, :],
                            in_=st.rearrange("p (c q) -> p c q", q=64))
                else:
                    xk, xoff = _piece(b, t // 2)
                    ih4 = xin[xk].rearrange("(c h r) q -> h r c q", h=2, r=64)
                    c0 = 4 * (t % 2)
                    for h in range(H_LOC):
                        st = spool.tile([64, 512], BF16, tag="st")
                        rbs = rb[h] if isinstance(rb, list) else \
                            rb[0:64, h * 512:(h + 1) * 512]
                        nc.vector.tensor_mul(
                            st, poc[0:64, h * 512:(h + 1) * 512], rbs)
                        dq.dma_start(
                            out=ih4[h, :, c0:c0 + 4, xoff:xoff + 128],
                            in_=st.rearrange("p (c q) -> p c q", q=128))
                step()

                if t == 1:
                    # exchange b = (b-1, h1) + (b, h0): fires mid-attention
                    emit_coll(b)
                elif t >= 2 and b == B - 1:
                    # final-piece quarters: t=2's exchange overlaps t=3's
                    # compute; only t=3's 128KB quarter is tail-exposed
                    nc.gpsimd.collective_compute(
                        "AllToAll", mybir.AluOpType.bypass,
                        replica_groups=[list(range(N_CORES))],
                        ins=[xq[t - 2].opt()], outs=[xqo[t - 2].opt()])
                    if t == 2:
                        # pre-load the tail oproj's first aT half under t=3
                        a31 = atp.tile([128, DC, 128], BF16, tag="aT",
                                       name="aT_tail")
                        nc.gpsimd.dma_start(
                            out=a31[:, :, 0:64],
                            in_=xqo[0].rearrange("(c p) q -> p c q", p=128))
                        pend_at[0] = a31

        # warm up the collective path while the pipeline head loads/projects
        warm_sb = consts.tile([N_CORES, 16], BF16)
        nc.scalar.dma_start(out=warm_sb, in_=xt[0:N_CORES, 0:16])
        nc.gpsimd.dma_start(out=warm_i[:, :], in_=warm_sb)
        nc.gpsimd.collective_compute(
            "AllToAll", mybir.AluOpType.bypass,
            replica_groups=[list(range(N_CORES))],
            ins=[warm_i.opt()], outs=[warm_o.opt()])

        kqv_tiles = {}
        pend_at = [None]
        t0 = proj_setup(0)
        xT0 = emit_xt_dma(0)
        # wo is only needed by oproj(0), well after proj(0): load it after
        # xT(0) so it doesn't delay the pipeline head on the sync DMA queue
        wo_sb = wpool.tile([128, DC, D], BF16, tag="wo_sb")
        nc.sync.dma_start(out=wo_sb, in_=wo.rearrange("p (c m) -> p c m", c=DC))

        # phase A only: attention(0)'s t=0/t=1 need just the first halves
        for _ in proj_steps(0, xT0, t0, [0], range(8)):
            pass
        for b in range(B):
            gens = []
            tail_gen = None
            if b == 0:
                # proj(0) phase B interleaves into attention(0)'s early
                # strips (done well before t=2 needs it)
                gens.append(proj_steps(0, xT0, t0, [1], range(8, 16)))
            if b + 1 < B:
                xTn = emit_xt_dma(b + 1)
                tn = proj_setup(b + 1)
                gens.append(proj_steps(b + 1, xTn, tn, [0, 1], range(16)))
            if b >= 1:
                # oproj halves land mid/late-batch: keeps PE work spread so
                # the HAM clock never sees a >3.4us idle window; half 0's
                # exchange completed during attention(b-1), half 1's
                # completes ~60% into this batch
                gens.append(_skip(6 if b == B - 1 else 0))
                gens.append(oproj_steps(b - 1, 0))
                gens.append(_skip(12))
                gens.append(oproj_steps(b - 1, 1))
            if b == B - 1:
                # first half of the last batch's oproj interleaves into the
                # final q-chunk's strips (its collective fired at t=1)
                tail_gen = _paced(oproj_steps(b, 0), 0.6)
            if b == 0:
                # phase B must beat attention(0)'s t=2 reads (yield ~22), but
                # proj(1) has until the batch end: pace them separately so
                # proj(1) fills the ACT-bound back half instead of bunching
                # into the front
                inter = chain(_paced(gens[0], 1.0), _paced(gens[1], 0.62))
            else:
                credit = 0.75 if b < B - 1 else 0.3
                inter = _paced(chain(*gens), credit) if gens else iter(())
            emit_attention(b, inter, tail_gen)
            for _ in inter:
                pass
            if tail_gen is not None:
                for _ in tail_gen:
                    pass
        # only the last 256KB exchange + 16 matmuls are exposed at the tail
        for _ in oproj_steps(B - 1, 1):
            pass

    nc.compile()
    return nc


_NC_CACHE = None


def _get_program():
    global _NC_CACHE
    if _NC_CACHE is None:
        _NC_CACHE = build_program()
    return _NC_CACHE


def _blk(w):
    """[D, M] -> [128, DC*M]: partition-major c-chunk blocking so the SBUF
    load is 128 contiguous lines."""
    m = w.shape[1]
    return np.ascontiguousarray(
        w.reshape(DC, 128, m).transpose(1, 0, 2).reshape(128, DC * m))


def _make_in_maps(x, w_qkv, b_qkv, w_o, b_o):
    x = np.asarray(x, dtype=np.float32).reshape(B * S, D)
    # [128, (b u c q)]: per-partition contiguous 16KB lines per half-batch
    xt = np.ascontiguousarray(
        x.T.reshape(DC, 128, B, 2, 1024).transpose(1, 2, 3, 0, 4).reshape(
            128, B * 2 * DC * 1024)).astype(BF16_NP)
    w_qkv = np.asarray(w_qkv, dtype=np.float32)
    b_qkv = np.asarray(b_qkv, dtype=np.float32)
    wo_bf = _blk(np.asarray(w_o, dtype=np.float32)).astype(BF16_NP)
    b_o = np.asarray(b_o, dtype=np.float32).reshape(1, D)
    in_maps = []
    for c in range(N_CORES):
        lo = c * HC
        hi = lo + HC
        in_maps.append({
            "xt": xt,
            "wq": _blk(w_qkv[:, lo:hi]).astype(BF16_NP),
            "wk": _blk(w_qkv[:, D + lo:D + hi]).astype(BF16_NP),
            "wv": _blk(w_qkv[:, 2 * D + lo:2 * D + hi]).astype(BF16_NP),
            "bq": np.ascontiguousarray(b_qkv[lo:hi].reshape(HC, 1)),
            "bk": np.ascontiguousarray(b_qkv[D + lo:D + hi].reshape(HC, 1)),
            "bv": np.ascontiguousarray(b_qkv[2 * D + lo:2 * D + hi].reshape(HC, 1)),
            "wo": wo_bf,
            "bo": b_o,
        })
    return in_maps


def _assemble(results):
    out = np.empty((B, S, D), dtype=np.float32)
    for c in range(N_CORES):
        for b in range(B):
            for h2 in range(2):
                r0 = b * 256 + 128 * h2
                blk = np.asarray(results[c]["out"][r0:r0 + 128],
                                 dtype=np.float32)
                if b == B - 1 and h2 == 1:
                    # final piece is exchanged per-t: core c owns 64 q-rows
                    # of each of t=2 and t=3
                    out[b, 1024 + 64 * c:1024 + 64 * c + 64, :] = blk[0:64]
                    out[b, 1536 + 64 * c:1536 + 64 * c + 64, :] = blk[64:128]
                else:
                    q0 = 1024 * h2 + 128 * c
                    out[b, q0:q0 + 128, :] = blk
    return out


def run(x, mask, w_qkv, b_qkv, w_o, b_o, trace=False, **trace_kwargs):
    """Run on hardware; returns (output, BassKernelResults)."""
    nc = _get_program()
    in_maps = _make_in_maps(x, w_qkv, b_qkv, w_o, b_o)
    res = run_bass_kernel_spmd(nc, in_maps, list(range(N_CORES)),
                               trace=trace, **trace_kwargs)
    return _assemble(res.results), res


def kernel(x, mask, w_qkv, b_qkv, w_o, b_o):
    out, _ = run(x, mask, w_qkv, b_qkv, w_o, b_o)
    return out
